# revision 56
# baseline (speedup 1.0000x reference)
"""Trainium2 Bass kernel for the BaseMemory coref scoring module.

Computes, for full inputs (M=65536 memory slots, D=768, E=20, H=64):
    score = relu(pair @ W1 + b1) @ W2 + b2, masked with ent_counter>0,
    where pair = [mem, ment, mem*ment, dist_emb, cnt_emb].

Sharding: data-parallel over the cluster dimension M across 8 NeuronCores.
Each core's shard of mem_vectors is laid out [D, MS] (contraction-major) so
the PE consumes it directly; all FLOPs and all HBM traffic stay on device.

Key folds (host side, O(D*H) + O(M) work on the small tensors only):
  - mem@W1_mem + (mem*ment)@W1_had = mem @ (W1_mem + diag(ment)@W1_had)
  - ment@W1_ment + b1 folded into the 10-row dist bucket table
  - bucket one-hots precomputed on host (O(M) int compares) and streamed
    as a [22, MS] bf16 plane; contracted on the PE against the folded
    10-row tables (masking folded into the PE accumulation, exact)
  - mem_vectors streamed as bf16: halves HBM traffic (the roofline term);
    all accumulation stays fp32 in PSUM
"""

import os
import numpy as np

# The bass kernel executes through the axon PJRT backend; make sure jax can
# see it even if the caller pinned JAX_PLATFORMS (e.g. to "cpu").
_jp = os.environ.get("JAX_PLATFORMS")
if _jp is not None and _jp != "" and "axon" not in _jp:
    os.environ["JAX_PLATFORMS"] = "axon," + _jp

M, D, E, H = 65536, 768, 20, 64
N_CORES = 8
MS = M // N_CORES          # rows per core = 8192
GROUP = 512                # rows per PE matmul group
N_GROUPS = MS // GROUP     # 16
SG = 4                     # groups per DMA super-group
N_SG = N_GROUPS // SG      # 4
KCH = D // 128             # 6 contraction chunks
NF = 22                    # 10 dist onehot, 10 cnt onehot, notmask, ones
HT = H + NF                # 86 rows of the score-matmul rhs
BIG = float(2 ** 14)       # pre-relu kill value for masked rows (fp16-exact)

_CACHE = {}


def _build():
    """Build + compile the 8-core SPMD bass program once per process."""
    if "nc" in _CACHE:
        return _CACHE["nc"]

    import concourse.bass as bass
    import concourse.mybir as mybir
    import concourse.tile as tile
    from concourse import bacc

    F32 = mybir.dt.float32
    BF16 = mybir.dt.bfloat16
    FP16 = mybir.dt.float16

    nc = bacc.Bacc("TRN2", target_bir_lowering=False, debug=False,
                   enable_asserts=False, num_devices=N_CORES)

    NPAIR = N_GROUPS // 2      # 8 column-pair blocks per core
    PB = 2 * GROUP             # 1024 columns per pair block
    # x pre-tiled on host as [pair, partition, kchunk, col]: each DMA moves
    # one pair block with a single contiguous 12KB line per partition
    xt_d = nc.dram_tensor("xt", [NPAIR, 128, KCH, PB], BF16,
                          kind="ExternalInput").ap()
    oh_d = nc.dram_tensor("oh", [NF, MS], FP16, kind="ExternalInput").ap()
    w1_d = nc.dram_tensor("w1", [D, H], BF16, kind="ExternalInput").ap()
    tcat_d = nc.dram_tensor("tcat", [NF, H], FP16, kind="ExternalInput").ap()
    wsc_d = nc.dram_tensor("wsc", [HT, 1], FP16, kind="ExternalInput").ap()
    out_d = nc.dram_tensor("out", [MS], F32, kind="ExternalOutput").ap()

    w1_r = w1_d.rearrange("(k p) n -> p k n", p=128)    # [128, 6, 64]
    out_r = out_d.rearrange("(s c) -> s c", s=N_SG)     # [4, 2048]
    oh_r = oh_d.rearrange("f (q c) -> f q c", q=NPAIR)  # [22, 8, 1024]

    relu = mybir.ActivationFunctionType.Relu

    with tile.TileContext(nc) as tc:
        with (
            tc.tile_pool(name="consts", bufs=1) as cpool,
            tc.tile_pool(name="xin", bufs=8) as px,
            tc.tile_pool(name="hts", bufs=8) as php,
            tc.tile_pool(name="osb", bufs=2) as posb,
            tc.tile_pool(name="psz", bufs=4, space="PSUM") as psz,
            tc.tile_pool(name="pss", bufs=4, space="PSUM") as pss,
        ):
            # consts issue on the scalar HWDGE queue so the big xt DMAs
            # (sync queue) start immediately
            w1t = cpool.tile([128, KCH, H], BF16, tag="w1t")
            nc.scalar.dma_start(w1t[:], w1_r[:])
            # tcat lives at base partition 64 so lhsT/rhs base partitions
            # match in the feature-accumulation matmul
            tcat_full = cpool.tile([HT, H], FP16, tag="tcat")
            tcat = tcat_full[H:HT, :]
            nc.scalar.dma_start(tcat, tcat_d[:])
            wsc = cpool.tile([HT, 1], FP16, tag="wsc")
            nc.scalar.dma_start(wsc[:], wsc_d[:])

            osb_tiles = {}
            from collections import deque
            pending = deque()

            def emit_score(g, htp, hoff, flush=False):
                # per-group score into its own 1-bank PSUM tile; 4-deep
                # pool so reuse never waits on a recent copy (WAR slack)
                sc = pss.tile([1, GROUP], F32, tag="pss")
                nc.tensor.matmul(sc[:], wsc[:], htp[:, hoff:hoff + GROUP],
                                 start=True, stop=True,
                                 skip_group_check=True)
                sq, j = divmod(g, SG)
                if j == 0:
                    osb_new = posb.tile([1, SG * GROUP], F32, tag="osb")
                    osb_tiles[sq] = osb_new
                orow = osb_tiles[sq][0:1, GROUP * j:GROUP * (j + 1)]
                if g % 2 == 0:
                    nc.scalar.copy(orow, sc[:])
                else:
                    nc.vector.tensor_copy(orow, sc[:])
                if j == SG - 1:
                    # the last super-group output rides the by-then idle
                    # sync HWDGE ring (lower fixed latency than SWDGE)
                    eng = nc.sync if sq == N_SG - 1 else nc.gpsimd
                    eng.dma_start(out_r[sq:sq + 1, :], osb_tiles.pop(sq)[:])

            # all pair-block loads issue upfront on the sync ring: DMA
            # streams at full rate in consumption order, one fat
            # contiguous descriptor per partition
            xqs = []
            for q in range(NPAIR):
                xq = px.tile([128, KCH, PB], BF16, tag="xin")
                if q == NPAIR - 1:
                    # last block lands in halves so only the final group's
                    # matmuls trail the last byte
                    nc.sync.dma_start(xq[:, :, 0:GROUP],
                                      xt_d[q][:, :, 0:GROUP])
                    nc.sync.dma_start(xq[:, :, GROUP:PB],
                                      xt_d[q][:, :, GROUP:PB])
                else:
                    nc.sync.dma_start(xq[:], xt_d[q])
                xqs.append(xq)

            # per-pair score-rhs tiles: rows 0..63 relu(z), rows 64..85 the
            # host-computed one-hot plane (own buffer per pair, so no
            # write-after-read false deps); all 8 loads issue upfront on
            # the scalar ring right after the consts
            htps = []
            for q in range(NPAIR):
                htp = php.tile([HT, PB], FP16, tag="hts")
                nc.scalar.dma_start(htp[H:HT, :], oh_r[:, q, :])
                htps.append(htp)

            for q in range(NPAIR):
                htp = htps[q]
                xq = xqs[q]
                # two groups per PSUM tile, computed on the two PE column
                # halves concurrently (tile_position inferred from the
                # output base partition)
                zt = psz.tile([2 * H, GROUP], F32, tag="psz")
                if q == NPAIR - 1:
                    # separate PSUM tile for the last pair's B group, so
                    # relu-A's (tile-granular) dependency does not include
                    # the B matmuls that wait on the final half-block DMA
                    ztb = psz.tile([2 * H, GROUP], F32, tag="psz")
                else:
                    ztb = zt
                # feature/bias/mask contribution first: it depends only on
                # the (early) one-hot DMA, so it fills PE idle time while
                # x streams in, and relu fires right after k==5
                nc.tensor.matmul(zt[0:H, :], tcat, htp[H:HT, 0:GROUP],
                                 start=True, stop=False,
                                 skip_group_check=True)
                nc.tensor.matmul(ztb[H:2 * H, :], tcat,
                                 htp[H:HT, GROUP:PB],
                                 start=True, stop=False,
                                 skip_group_check=True)
                if q == NPAIR - 1:
                    # last pair: finish the A group on the first half-block
                    # DMA before any B matmul can block the PE FIFO on the
                    # second half-block
                    for k in range(KCH):
                        nc.tensor.matmul(zt[0:H, :], w1t[:, k, :],
                                         xq[:, k, 0:GROUP],
                                         start=False, stop=(k == KCH - 1),
                                         skip_group_check=True)
                    for k in range(KCH):
                        nc.tensor.matmul(ztb[H:2 * H, :], w1t[:, k, :],
                                         xq[:, k, GROUP:PB],
                                         start=False, stop=(k == KCH - 1),
                                         skip_group_check=True)
                else:
                    for k in range(KCH):
                        nc.tensor.matmul(zt[0:H, :], w1t[:, k, :],
                                         xq[:, k, 0:GROUP],
                                         start=False, stop=(k == KCH - 1),
                                         skip_group_check=True)
                        nc.tensor.matmul(ztb[H:2 * H, :], w1t[:, k, :],
                                         xq[:, k, GROUP:PB],
                                         start=False, stop=(k == KCH - 1),
                                         skip_group_check=True)

                # relus run concurrently on ACT and DVE
                nc.scalar.activation(htp[0:H, 0:GROUP], zt[0:H, :], relu)
                nc.vector.tensor_scalar_max(htp[0:H, GROUP:PB],
                                            ztb[H:2 * H, :], 0.0)
                pending.append((2 * q, htp, 0))
                pending.append((2 * q + 1, htp, GROUP))
                # scores trail by two pairs so they never stall the PE
                # queue waiting on a relu
                while len(pending) >= 5:
                    emit_score(*pending.popleft())
            while pending:
                emit_score(*pending.popleft(), flush=True)

    nc.compile()
    _CACHE["nc"] = nc
    return nc


def _bucket(c):
    """Reference get_bucket, replicated with the same XLA CPU float ops so
    boundary cases (c = 8, 16, 32) bucket identically."""
    import math
    import jax
    import jax.numpy as jnp
    cpu = jax.devices("cpu")[0]
    with jax.default_device(cpu):
        c = jnp.asarray(c).astype(jnp.int32)
        logspace = jnp.floor(
            jnp.log(jnp.maximum(c, 1).astype(jnp.float32)) / math.log(2)
        ).astype(jnp.int32) + 3
        idx = jnp.where(c <= 4, c, logspace)
        return np.asarray(jnp.clip(idx, 0, 9))


def _prepare_maps(ment_emb, mem_vectors, dist_table, counter_table,
                  W1, b1, W2, b2, ent_counter, last_mention_start, ment_start):
    import ml_dtypes
    f32 = np.float32
    bf16 = ml_dtypes.bfloat16
    fp16 = np.float16
    ment = np.asarray(ment_emb, f32)
    mem = np.asarray(mem_vectors, f32)
    W1 = np.asarray(W1, f32)
    ms = int(np.asarray(ment_start))

    W1m, W1r, W1h = W1[0:D], W1[D:2 * D], W1[2 * D:3 * D]
    W1d, W1c = W1[3 * D:3 * D + E], W1[3 * D + E:3 * D + 2 * E]

    w1eff = (W1m + ment[:, None] * W1h).astype(f32)              # [768, 64]
    bias_vec = (np.asarray(b1, f32) + ment @ W1r).astype(f32)    # [64]
    T_d = (np.asarray(dist_table, f32) @ W1d + bias_vec).astype(f32)
    T_c = (np.asarray(counter_table, f32) @ W1c).astype(f32)
    b2v = float(np.asarray(b2, f32).reshape(-1)[0])

    tcat = np.concatenate(
        [T_d, T_c, np.full((1, H), -BIG, f32), np.zeros((1, H), f32)], 0)
    # single score matmul: rows 0..63 act on relu(z), rows 64..85 on onehot
    wsc = np.zeros((HT, 1), f32)
    wsc[0:H, 0] = np.asarray(W2, f32).reshape(-1)
    wsc[H + 20, 0] = -10000.0 - b2v
    wsc[H + 21, 0] = b2v

    cnt_i = np.asarray(ent_counter).astype(np.int64)
    dist_i = ms - np.asarray(last_mention_start).astype(np.int64)
    bd = _bucket(dist_i)                                         # [M] in 0..9
    bc = _bucket(cnt_i)                                          # [M] in 0..9
    r = np.arange(10)
    oh = np.empty((NF, M), f32)
    oh[0:10] = (bd[None, :] == r[:, None])
    oh[10:20] = (bc[None, :] == r[:, None])
    oh[20] = (cnt_i <= 0)
    oh[21] = 1.0
    oh = oh.astype(fp16)

    w1_b = w1eff.astype(bf16)
    tcat_b = tcat.astype(fp16)
    wsc_b = wsc.astype(fp16)

    in_maps = []
    for c in range(N_CORES):
        sl = slice(c * MS, (c + 1) * MS)
        # [pair, partition, kchunk, col] pre-tiling: one contiguous 12KB
        # line per partition per pair-block DMA
        xt = (mem[sl].T.astype(bf16)
              .reshape(KCH, 128, MS // 1024, 1024)
              .transpose(2, 1, 0, 3))
        in_maps.append(dict(
            xt=np.ascontiguousarray(xt),
            oh=np.ascontiguousarray(oh[:, sl]),
            w1=w1_b, tcat=tcat_b, wsc=wsc_b))
    return in_maps


def _postprocess(results):
    out = np.empty(M + 1, np.float32)
    for c in range(N_CORES):
        out[c * MS:(c + 1) * MS] = results[c]["out"]
    out[M] = 0.0
    return out


def run_spmd(in_maps, trace=False):
    from concourse.bass_utils import run_bass_kernel_spmd
    nc = _build()
    return run_bass_kernel_spmd(nc, in_maps, list(range(N_CORES)), trace=trace)


def kernel(**inputs):
    in_maps = _prepare_maps(**inputs)
    res = run_spmd(in_maps, trace=False)
    return _postprocess(res.results)
